# revision 3
# baseline (speedup 1.0000x reference)
"""BiGRU Trainium2 kernel, 8-core SPMD.

Strategy: shard the hidden dimension H=1024 8 ways (128 per core). Each core
computes its 128-wide slice of both GRU directions for the full batch; the
per-step hidden state is exchanged between all cores with SWDGE remote DMA
(SBUF -> SBUF, one receive slot per peer). The recurrence matmul is
hidden-state-stationary: lhsT = h^T tiles (K = H on partitions), rhs = Whh^T
column slices, so the PE streams weight columns at full rate; fwd and bwd
directions run concurrently on the two halves of the PE array (out partition
base 0 / 64).

The input projection xg = x @ Wih^T + biases (both directions) is computed
on-device, interleaved with the recurrence to fill PE idle time, and staged
through DRAM in [t*B + b] row order so each step loads contiguous tiles.

SPMD twist: remote-DMA relative destinations XOR the *physical* NC index and
instruction streams are identical on all cores, so per-core differences live
in data only. Receive slot d on logical core r holds the h-slice of core
sigma_r(d) = FINV[F[r] ^ d] (F = logical->physical NC map); the host permutes
each core's Whh^T / W_lin^T contraction blocks by sigma_r so one static slot
order is correct everywhere.
"""

import os
import sys

sys.path.insert(0, "/opt/trn_rl_repo")

import numpy as np
import ml_dtypes

import concourse.bass as bass
import concourse.mybir as mybir

# ---- problem constants -------------------------------------------------------
B = 64  # batch
T = 512  # sequence length
I = 1024  # input features
H = 1024  # hidden
O = 1024  # output features
N = 8  # cores
KT = 8  # 128-row contraction blocks in H (and I)
P = 128
SL = 128  # per-core H slice
G3 = 3 * SL  # per-core gate columns (r|z|n)

# logical -> physical NC map of this fabric (measured); relative XOR remote-DMA
# routing operates in physical space.
F_MAP = [0, 1, 2, 3, 6, 7, 4, 5]
FINV = [F_MAP.index(i) for i in range(8)]

BF16 = mybir.dt.bfloat16
F32 = mybir.dt.float32
AFT = mybir.ActivationFunctionType


def sigma(r: int, d: int) -> int:
    """H-slice owner whose tile lands in receive slot d on logical core r."""
    return FINV[F_MAP[r] ^ d]


# ---- device program ----------------------------------------------------------


def build_program(t_steps: int = T):
    """One SPMD Bacc program, identical for all 8 cores.

    t_steps must be even; the phase-1 token tiling assumes
    n_tok_tiles = t_steps / 2 (each tile = 2 t-values x 64 batch rows).
    """
    from concourse.bacc import Bacc

    assert t_steps % 2 == 0
    NTT = t_steps // 2  # phase-1 token tiles
    PRO = min(4, NTT)  # tiles processed before step 0
    XPF = 4  # xg prefetch depth (ring)

    DBG = os.environ.get("KDBG", "").split(",")
    no_bcast = "nobcast" in DBG
    no_epi = "noepi" in DBG
    no_rec = "norec" in DBG
    no_gates = "nogates" in DBG
    no_transp = "notransp" in DBG
    no_mmrec = "nommrec" in DBG
    act_only = "actonly" in DBG
    no_tanh = "notanh" in DBG
    no_dvemix = "nodvemix" in DBG

    nc = Bacc()

    # -- IO -------------------------------------------------------------------
    xT = nc.declare_dram_parameter("xT", [NTT, P, KT * P], BF16, isOutput=False)
    wih = nc.declare_dram_parameter("wih", [KT, P, 2 * G3], BF16, isOutput=False)
    whh = nc.declare_dram_parameter("whh", [KT, P, 2 * G3], BF16, isOutput=False)
    wlin = nc.declare_dram_parameter("wlin", [2 * KT, P, SL], BF16, isOutput=False)
    bias1 = nc.declare_dram_parameter("bias1", [1, 2 * G3], BF16, isOutput=False)
    biasn = nc.declare_dram_parameter("biasn", [1, 2 * SL], BF16, isOutput=False)
    blin = nc.declare_dram_parameter("blin", [1, SL], BF16, isOutput=False)
    ident = nc.declare_dram_parameter("ident", [P, P], BF16, isOutput=False)
    ones = nc.declare_dram_parameter("ones", [1, P], BF16, isOutput=False)
    out = nc.declare_dram_parameter("out", [B, SL], F32, isOutput=True)

    # phase-1 output staging through DRAM, [t*64 + b, 384] row order
    xgf_d = nc.dram_tensor("xgf_d", [t_steps * B, G3], BF16)
    xgb_d = nc.dram_tensor("xgb_d", [t_steps * B, G3], BF16)

    n_init_dma = KT + KT + 2 * KT + 5  # whh, wih, wlin blocks + 5 small consts

    def ph1_tile(p: int) -> int:
        """phase-1 processing order: ends inward (0, NTT-1, 1, NTT-2, ...)."""
        return p // 2 if p % 2 == 0 else NTT - 1 - p // 2

    from contextlib import ExitStack

    es = ExitStack()
    with es:
        sem = lambda name: es.enter_context(nc.semaphore(name))
        sbuf = lambda name, shape, dt=BF16: es.enter_context(
            nc.sbuf_tensor(name, shape, dt)
        )
        psum = lambda name, shape, dt: es.enter_context(nc.psum_tensor(name, shape, dt))

        block = es.enter_context(nc.Block())
        init_sem = sem("init_sem")
        hz_sem = sem("hz_sem")
        bar_sem = sem("bar_sem")
        bar_p = sem("bar_p")
        bar_l = sem("bar_l")
        rsem = [[sem(f"rsem{par}_{d}") for d in range(N)] for par in range(2)]
        lsem = [sem("lsem0"), sem("lsem1")]
        prep_sem = sem("prep_sem")
        psum_rdy = sem("psum_rdy")
        a2v_r = sem("a2v_r")
        a2v_z = sem("a2v_z")
        a2v_n = sem("a2v_n")
        v2a_np = sem("v2a_np")
        pf_v = sem("pf_v")
        v2p = sem("v2p")
        vch = sem("vch")
        p2v = sem("p2v")
        tdone = sem("tdone")
        xg_dma = [sem(f"xg_dma{i}") for i in range(XPF)]
        xgc_p = sem("xgc_p")
        xt_dma = [sem("xt_dma0"), sem("xt_dma1")]
        p1_rdy = sem("p1_rdy")
        p1_cp = sem("p1_cp")
        p1_w = [sem("p1_w0"), sem("p1_w1")]
        fin_sem = sem("fin_sem")

        whh_s = sbuf("whh_s", [P, KT * 2 * G3])
        wih_s = sbuf("wih_s", [P, KT * 2 * G3])
        wlin_s = sbuf("wlin_s", [P, 2 * KT * SL])
        hbuf = sbuf("hbuf", [P, 2 * N * P])
        xg_s = sbuf("xg_s", [P, XPF * G3])
        xt_s = sbuf("xt_s", [P, 2 * KT * P])
        rz_s = sbuf("rz_s", [P, 2 * SL])
        t1_s = sbuf("t1_s", [P, SL])
        npre_s = sbuf("npre_s", [P, SL])
        n_s = sbuf("n_s", [P, SL])
        s1_s = sbuf("s1_s", [P, SL])
        s2_s = sbuf("s2_s", [P, SL])
        hst_s = sbuf("hst_s", [P, SL])
        hgn_s = sbuf("hgn_s", [P, SL])
        tb_s = sbuf("tb_s", [P, 2 * P])
        xgf_st = sbuf("xgf_st", [P, 2 * G3])
        xgb_st = sbuf("xgb_st", [P, 2 * G3])
        ident_s = sbuf("ident_s", [P, P])
        ones_s = sbuf("ones_s", [1, P])
        bias1_s = sbuf("bias1_s", [1, 2 * G3])
        biasn_s = sbuf("biasn_s", [1, 2 * SL])
        blin_s = sbuf("blin_s", [1, SL])
        out_s = sbuf("out_s", [B, SL], F32)
        # separate tensors so double-buffers land in different PSUM banks
        # (PE-write + DVE-read of one bank is a hardware fault)
        ps_rec0 = psum("ps_rec0", [P, G3], F32)
        ps_rec1 = psum("ps_rec1", [P, G3], F32)
        ps_t0 = psum("ps_t0", [P, P], BF16)
        ps_t1 = psum("ps_t1", [P, P], BF16)
        ps_p1f = psum("ps_p1f", [P, G3], F32)
        ps_p1b = psum("ps_p1b", [P, G3], F32)
        ps_rec = [ps_rec0, ps_rec1]
        ps_t = [ps_t0, ps_t1]

        def hb(t):
            """hbuf column offset of the buffer read at step t."""
            return (t % 2) * N * P

        # ---------------- SYNC: all HWDGE DMA traffic ---------------------
        @block.sync
        def _(s):
            for k in range(KT):
                s.dma_start(
                    out=whh_s[:, k * 2 * G3 : (k + 1) * 2 * G3], in_=whh[k, :, :]
                ).then_inc(init_sem, 16)
                s.dma_start(
                    out=wih_s[:, k * 2 * G3 : (k + 1) * 2 * G3], in_=wih[k, :, :]
                ).then_inc(init_sem, 16)
            for k in range(2 * KT):
                s.dma_start(
                    out=wlin_s[:, k * SL : (k + 1) * SL], in_=wlin[k, :, :]
                ).then_inc(init_sem, 16)
            s.dma_start(out=ident_s[:, :], in_=ident[:, :]).then_inc(init_sem, 16)
            s.dma_start(out=ones_s[:, :], in_=ones[:, :]).then_inc(init_sem, 16)
            s.dma_start(out=bias1_s[:, :], in_=bias1[:, :]).then_inc(init_sem, 16)
            s.dma_start(out=biasn_s[:, :], in_=biasn[:, :]).then_inc(init_sem, 16)
            s.dma_start(out=blin_s[:, :], in_=blin[:, :]).then_inc(init_sem, 16)

            def load_xt(p):
                if p >= NTT:
                    return
                if p >= 2:
                    s.wait_ge(p1_rdy, p - 1)  # xt ring slot free
                s.dma_start(
                    out=xt_s[:, (p % 2) * KT * P : ((p % 2) + 1) * KT * P],
                    in_=xT[ph1_tile(p), :, :],
                ).then_inc(xt_dma[p % 2], 16)

            def write_ph1(p):
                if p >= NTT:
                    return
                tau = ph1_tile(p)
                s.wait_ge(p1_cp, 2 * (p + 1))
                s.dma_start(
                    out=xgf_d[2 * tau * B : 2 * tau * B + P, :],
                    in_=xgf_st[:, (p % 2) * G3 : (p % 2) * G3 + G3],
                ).then_inc(p1_w[p % 2], 16)
                s.dma_start(
                    out=xgb_d[2 * tau * B : 2 * tau * B + P, :],
                    in_=xgb_st[:, (p % 2) * G3 : (p % 2) * G3 + G3],
                ).then_inc(p1_w[p % 2], 16)

            def load_xg(t):
                if no_rec or t >= t_steps or t < 0:
                    return
                if load_xg.done >= t + 1:
                    return
                load_xg.done = t + 1
                # phase-1 tiles 0..need_p-1 cover fwd row t and bwd row T-1-t
                need_p = min(2 * (t // 2) + 2, NTT)
                s.wait_ge(p1_w[0], 32 * (need_p - need_p // 2))
                s.wait_ge(p1_w[1], 32 * (need_p // 2))
                if t >= XPF:
                    s.wait_ge(v2a_np, t - XPF + 1)
                    s.wait_ge(xgc_p, t - XPF + 1)
                slot = (t % XPF) * G3
                s.dma_start(
                    out=xg_s[0:B, slot : slot + G3],
                    in_=xgf_d[t * B : (t + 1) * B, :],
                ).then_inc(xg_dma[t % XPF], 16)
                s.dma_start(
                    out=xg_s[B:P, slot : slot + G3],
                    in_=xgb_d[(t_steps - 1 - t) * B : (t_steps - t) * B, :],
                ).then_inc(xg_dma[t % XPF], 16)

            # prologue: interleave so no FIFO head-of-line cycle forms
            # (load_xt(p+2) transitively needs write_ph1(p-2) through PE/DVE)
            for p in range(4):
                load_xt(p)
            write_ph1(0)
            load_xt(4)
            write_ph1(1)
            load_xt(5)
            write_ph1(2)
            write_ph1(3)
            load_xg.done = 0
            for t in range(XPF):
                load_xg(t)
            for t in range(t_steps):
                write_ph1(PRO + t)
                load_xt(PRO + t + 2)
                load_xg(t + XPF - 1)

            s.wait_ge(fin_sem, 1)
            s.dma_start(out=out[:, :], in_=out_s[:, :]).then_inc(fin_sem, 16)

        # ---------------- PE: matmuls, transpose, phase-1 ------------------
        @block.tensor
        def _(pe):
            def ph1_work(p):
                if p >= NTT:
                    return
                pe.wait_ge(xt_dma[p % 2], 16 * (p // 2 + 1))
                if p >= 1:
                    pe.wait_ge(p1_cp, 2 * p)  # psum consumed by DVE copies
                xo = (p % 2) * KT * P
                for k in range(KT):
                    lt = xt_s[:, xo + k * P : xo + (k + 1) * P]
                    pe.matmul(
                        ps_p1f[:, :],
                        lt,
                        wih_s[:, k * 2 * G3 : k * 2 * G3 + G3],
                        start=(k == 0),
                        stop=False,
                    )
                    pe.matmul(
                        ps_p1b[:, :],
                        lt,
                        wih_s[:, k * 2 * G3 + G3 : (k + 1) * 2 * G3],
                        start=(k == 0),
                        stop=False,
                    )
                pe.matmul(
                    ps_p1f[:, :],
                    ones_s[0:1, :],
                    bias1_s[0:1, 0:G3],
                    start=False,
                    stop=True,
                )
                pe.matmul(
                    ps_p1b[:, :],
                    ones_s[0:1, :],
                    bias1_s[0:1, G3 : 2 * G3],
                    start=False,
                    stop=True,
                ).then_inc(p1_rdy, 1)

            pe.wait_ge(init_sem, 16 * n_init_dma)
            pe.wait_ge(hz_sem, 2)
            for p in range(PRO):
                ph1_work(p)

            for t in range(t_steps):
                ps = ps_rec[t % 2]
                if no_rec:
                    ph1_work(PRO + t)
                    continue
                if t >= 1 and not no_bcast:
                    for d in range(N):
                        pe.wait_ge(rsem[(t - 1) % 2][d], 2 * ((t - 1) // 2 + 1))
                if t >= 2:
                    pe.wait_ge(a2v_z, 2 * (t - 1))
                    pe.wait_ge(pf_v, t - 1)
                pe.wait_ge(xg_dma[t % XPF], 32 * (t // XPF + 1))
                hbo = hb(t)
                slot = (t % XPF) * G3
                if no_mmrec:
                    pe.matmul(
                        ps[:, 0 : 2 * SL],
                        ident_s[:, :],
                        xg_s[:, slot : slot + 2 * SL],
                        start=True,
                        stop=True,
                    ).then_inc(xgc_p, 1)
                    pe.matmul(
                        ps[0:B, 2 * SL : G3],
                        ones_s[0:1, 0:B],
                        biasn_s[0:1, 0:SL],
                        start=True,
                        stop=True,
                    )
                    pe.matmul(
                        ps[B:P, 2 * SL : G3],
                        ones_s[0:1, B:P],
                        biasn_s[0:1, SL : 2 * SL],
                        start=True,
                        stop=True,
                        skip_group_check=True,
                    ).then_inc(psum_rdy, 1)
                if not no_mmrec:
                    # d=0 opens the accumulation (start marks the whole bank
                    # pending-zero per partition half); d=7 closes it; the xg and
                    # bhh_n matmuls then accumulate on top (group-check skipped:
                    # their region is a slice of the already-opened groups).
                    for d in range(N):
                        pe.matmul(
                            ps[0:B, :],
                            hbuf[:, hbo + d * P : hbo + d * P + B],
                            whh_s[:, d * 2 * G3 : d * 2 * G3 + G3],
                            start=(d == 0),
                            stop=(d == N - 1),
                        )
                        pe.matmul(
                            ps[B:P, :],
                            hbuf[:, hbo + d * P + B : hbo + (d + 1) * P],
                            whh_s[:, d * 2 * G3 + G3 : (d + 1) * 2 * G3],
                            start=(d == 0),
                            stop=(d == N - 1),
                            skip_group_check=True,
                        )
                    pe.matmul(
                        ps[:, 0 : 2 * SL],
                        ident_s[:, :],
                        xg_s[:, slot : slot + 2 * SL],
                        start=False,
                        stop=False,
                        skip_group_check=True,
                    ).then_inc(xgc_p, 1)
                    pe.matmul(
                        ps[0:B, 2 * SL : G3],
                        ones_s[0:1, 0:B],
                        biasn_s[0:1, 0:SL],
                        start=False,
                        stop=False,
                        skip_group_check=True,
                    )
                    pe.matmul(
                        ps[B:P, 2 * SL : G3],
                        ones_s[0:1, B:P],
                        biasn_s[0:1, SL : 2 * SL],
                        start=False,
                        stop=False,
                        skip_group_check=True,
                    ).then_inc(psum_rdy, 1)

                # phase-1 fill while the gates run on ACT/DVE
                ph1_work(PRO + t)

                # transpose h_new into the broadcast source layout
                if not no_transp:
                    pe.wait_ge(v2p, t + 1)
                    pe.transpose(ps_t[t % 2][:, :], hst_s[:, :], ident_s[:, :]).then_inc(
                        p2v, 1
                    )

            if no_epi or no_rec:
                pe.wait_ge(p1_cp, 2 * NTT)
                pe.matmul(
                    ps_p1f[0:B, 0:SL],
                    ident_s[:, 0:B],
                    blin_s[0:1, :] if False else wlin_s[0:128, 0:SL],
                    start=True,
                    stop=True,
                ).then_inc(psum_rdy, 1 if no_rec else t_steps + 1)
                return

            # final linear: out = [h_fwd | h_bwd] @ W_lin^T + b_lin
            if not no_bcast:
                for d in range(N):
                    pe.wait_ge(
                        rsem[(t_steps - 1) % 2][d], 2 * ((t_steps - 1) // 2 + 1)
                    )
            pe.wait_ge(p1_cp, 2 * NTT)  # ps_p1f free
            hbo = hb(t_steps)
            for d in range(N):
                pe.matmul(
                    ps_p1f[0:B, 0:SL],
                    hbuf[:, hbo + d * P : hbo + d * P + B],
                    wlin_s[:, d * SL : (d + 1) * SL],
                    start=(d == 0),
                    stop=False,
                )
            for d in range(N):
                pe.matmul(
                    ps_p1f[0:B, 0:SL],
                    hbuf[:, hbo + d * P + B : hbo + (d + 1) * P],
                    wlin_s[:, (N + d) * SL : (N + d + 1) * SL],
                    start=False,
                    stop=False,
                )
            pe.matmul(
                ps_p1f[0:B, 0:SL],
                ones_s[0:1, 0:B],
                blin_s[0:1, :],
                start=False,
                stop=True,
            ).then_inc(psum_rdy, 1)

        # ---------------- ACT: sigmoids + tanh ----------------------------
        @block.scalar
        def _(a):
            if no_rec or no_gates:
                return
            for t in range(t_steps):
                ps = ps_rec[t % 2]
                a.wait_ge(psum_rdy, t + 1)
                a.activation(rz_s[:, 0:SL], ps[:, 0:SL], AFT.Sigmoid).then_inc(
                    a2v_r, 1
                )
                a.activation(
                    rz_s[:, SL : 2 * SL], ps[:, SL : 2 * SL], AFT.Sigmoid
                ).then_inc(a2v_z, 1)
                a.activation(hgn_s[:, :], ps[:, 2 * SL : G3], AFT.Copy).then_inc(
                    a2v_z, 1
                )
                if act_only or no_tanh:
                    a.activation(n_s[:, :], npre_s[:, :], AFT.Sigmoid).then_inc(
                        a2v_n, 1
                    )
                else:
                    a.wait_ge(v2a_np, t + 1)
                    a.activation(n_s[:, :], npre_s[:, :], AFT.Tanh).then_inc(a2v_n, 1)

        # ---------------- DVE: gate arithmetic, copies --------------------
        @block.vector
        def _(v):
            v.memset(hbuf[:, :], 0.0).then_inc(hz_sem, 1)
            v.memset(hst_s[:, :], 0.0).then_inc(hz_sem, 1)
            v.wait_ge(hz_sem, 2)

            def ph1_copy(p):
                if p >= NTT:
                    return
                v.wait_ge(p1_rdy, p + 1)
                if p >= 2:
                    v.wait_ge(p1_w[p % 2], 32 * (p // 2))  # staging slot free
                v.tensor_copy(
                    xgf_st[:, (p % 2) * G3 : (p % 2) * G3 + G3], ps_p1f[:, :]
                )
                v.tensor_copy(
                    xgb_st[:, (p % 2) * G3 : (p % 2) * G3 + G3], ps_p1b[:, :]
                ).then_inc(p1_cp, 2)

            for p in range(PRO):
                ph1_copy(p)

            for t in range(t_steps):
                if no_rec:
                    ph1_copy(PRO + t)
                    continue
                ps = ps_rec[t % 2]
                slot = (t % XPF) * G3
                if no_gates:
                    v.wait_ge(psum_rdy, t + 1)
                    v.tensor_copy(s1_s[:, :], hst_s[:, :]).then_inc(v2p, 1)
                    if not no_transp:
                        v.wait_ge(p2v, t + 1)
                        v.tensor_copy(
                            tb_s[:, (t % 2) * P : (t % 2) * P + P], ps_t[t % 2][:, :]
                        ).then_inc(tdone, 1)
                    ph1_copy(PRO + t)
                    continue
                v.wait_ge(a2v_r, t + 1)
                if act_only:
                    v.tensor_copy(s1_s[:, :], hst_s[:, :]).then_inc(v2p, 1)
                    if not no_transp:
                        v.wait_ge(p2v, t + 1)
                        v.tensor_copy(
                            tb_s[:, (t % 2) * P : (t % 2) * P + P], ps_t[t % 2][:, :]
                        ).then_inc(tdone, 1)
                    ph1_copy(PRO + t)
                    continue
                # t1 = r * hg_n (hg_n staged through SBUF by ACT: a DVE
                # TensorTensor read of PSUM hard-faults this device)
                v.wait_ge(a2v_z, 2 * t + 2)
                v.tensor_mul(t1_s[:, :], rz_s[:, 0:SL], hgn_s[:, :]).then_inc(
                    pf_v, 1
                )
                # n_pre = t1 + xg_n
                v.wait_ge(pf_v, t + 1)  # t1 writeback drained
                v.wait_ge(xg_dma[t % XPF], 32 * (t // XPF + 1))
                v.tensor_add(
                    npre_s[:, :], t1_s[:, :], xg_s[:, slot + 2 * SL : slot + G3]
                ).then_inc(v2a_np, 1)
                v.wait_ge(a2v_n, t + 1)
                # h_new = n + z*(h - n)
                if t >= 1:
                    v.wait_ge(v2p, t)  # prior h_new writeback drained
                v.tensor_sub(s1_s[:, :], hst_s[:, :], n_s[:, :]).then_inc(vch, 1)
                v.wait_ge(a2v_z, 2 * t + 1)
                v.wait_ge(vch, 2 * t + 1)
                v.tensor_mul(s2_s[:, :], rz_s[:, SL : 2 * SL], s1_s[:, :]).then_inc(
                    vch, 1
                )
                v.wait_ge(vch, 2 * t + 2)
                v.tensor_add(hst_s[:, :], n_s[:, :], s2_s[:, :]).then_inc(v2p, 1)

                # move the transposed tile into the broadcast source buffer
                if not no_transp:
                    v.wait_ge(p2v, t + 1)
                    if t >= 2 and not no_bcast:
                        v.wait_ge(lsem[t % 2], 128 * (t // 2))  # t-2 sends done
                    v.tensor_copy(
                        tb_s[:, (t % 2) * P : (t % 2) * P + P], ps_t[t % 2][:, :]
                    ).then_inc(tdone, 1)

                ph1_copy(PRO + t)

            v.wait_ge(psum_rdy, 1 if no_rec else t_steps + 1)
            v.tensor_copy(out_s[:, :], ps_p1f[0:B, 0:SL]).then_inc(fin_sem, 1)

        # ---------------- GPSIMD: remote broadcasts ------------------------
        @block.gpsimd
        def _(g):
            if no_bcast or no_rec:
                return
            # start barrier: no core may broadcast into peers' hbuf until every
            # core has zero-initialized its own hbuf.
            g.wait_ge(hz_sem, 1)
            g.remote_sem_update_broadcast(
                remote_sem=bar_sem,
                local_sem=bar_l,
                rdests=[(0, k) for k in range(N)],
            ).then_inc(bar_p, 1)
            g.wait_ge(bar_p, 1)
            g.trigger_dma(count=1)
            g.wait_ge(bar_sem, 16)
            # barrier passed: every core has zeroed hbuf, so peers' step-0
            # broadcasts may now arrive at any time.

            for t in range(t_steps):
                # order the desc-gen after this step's matmuls: the remote
                # writes must be provably after every receiver's step t-1
                # reads, a chain that runs through our rsem waits.
                g.wait_ge(psum_rdy, t + 1)
                for d in range(N):
                    rd = [None] * N
                    rd[d] = (0, d)
                    g.remote_dma_broadcast(
                        out_ap=hbuf[:, hb(t + 1) + d * P : hb(t + 1) + (d + 1) * P],
                        in_ap=tb_s[:, (t % 2) * P : (t % 2) * P + P],
                        remote_sem=rsem[t % 2][d],
                        local_sem=lsem[t % 2],
                        rdests=rd,
                    ).then_inc(prep_sem, 1)
                g.wait_ge(prep_sem, N * (t + 1))
                g.wait_ge(tdone, t + 1)
                g.trigger_dma(count=N)

    nc.finalize()
    return nc


# ---- host-side input preparation ---------------------------------------------

BF16_NP = ml_dtypes.bfloat16


def _own_rows(r: int) -> np.ndarray:
    """Row indices (into 3H) of core r's r/z/n gate slices."""
    base = np.arange(r * SL, (r + 1) * SL)
    return np.concatenate([base, H + base, 2 * H + base])


def make_core_inputs(
    r,
    xT_shared,
    Wih_f,
    Whh_f,
    bih_f,
    bhh_f,
    Wih_b,
    Whh_b,
    bih_b,
    bhh_b,
    W_lin,
    b_lin,
):
    rows = _own_rows(r)
    perm = [sigma(r, d) for d in range(N)]

    def wih_pack():
        wf = np.ascontiguousarray(Wih_f[rows, :].T)  # [I, 384]
        wb = np.ascontiguousarray(Wih_b[rows, :].T)
        o = np.empty((KT, P, 2 * G3), dtype=BF16_NP)
        for k in range(KT):
            o[k, :, 0:G3] = wf[k * P : (k + 1) * P, :]
            o[k, :, G3 : 2 * G3] = wb[k * P : (k + 1) * P, :]
        return o

    def whh_pack():
        wf = np.ascontiguousarray(Whh_f[rows, :].T)  # [H, 384]
        wb = np.ascontiguousarray(Whh_b[rows, :].T)
        o = np.empty((KT, P, 2 * G3), dtype=BF16_NP)
        for d in range(N):
            s = perm[d]
            o[d, :, 0:G3] = wf[s * P : (s + 1) * P, :]
            o[d, :, G3 : 2 * G3] = wb[s * P : (s + 1) * P, :]
        return o

    def wlin_pack():
        wl = np.ascontiguousarray(W_lin[r * SL : (r + 1) * SL, :].T)  # [2H, 128]
        o = np.empty((2 * KT, P, SL), dtype=BF16_NP)
        for d in range(N):
            s = perm[d]
            o[d] = wl[s * P : (s + 1) * P, :]
            o[N + d] = wl[H + s * P : H + (s + 1) * P, :]
        return o

    brz_f = (bih_f + bhh_f)[rows]
    brz_b = (bih_b + bhh_b)[rows]
    b1 = np.empty((1, 2 * G3), dtype=BF16_NP)
    b1[0, 0 : 2 * SL] = brz_f[0 : 2 * SL]
    b1[0, 2 * SL : G3] = bih_f[rows][2 * SL : G3]
    b1[0, G3 : G3 + 2 * SL] = brz_b[0 : 2 * SL]
    b1[0, G3 + 2 * SL : 2 * G3] = bih_b[rows][2 * SL : G3]

    bn = np.empty((1, 2 * SL), dtype=BF16_NP)
    bn[0, 0:SL] = bhh_f[rows][2 * SL : G3]
    bn[0, SL : 2 * SL] = bhh_b[rows][2 * SL : G3]

    return {
        "xT": xT_shared,
        "wih": wih_pack(),
        "whh": whh_pack(),
        "wlin": wlin_pack(),
        "bias1": b1,
        "biasn": bn,
        "blin": b_lin[r * SL : (r + 1) * SL].reshape(1, SL).astype(BF16_NP),
        "ident": np.eye(P, dtype=BF16_NP),
        "ones": np.ones((1, P), dtype=BF16_NP),
    }


def make_xT(input_btI: np.ndarray, t_steps: int = T) -> np.ndarray:
    """[B,T,I] -> [NTT, P, KT*P] bf16, token order (k, t_off, b) in the free dim."""
    ntt = t_steps // 2
    xt = np.transpose(input_btI, (1, 0, 2))  # [T, B, I]
    v = xt.reshape(ntt, 2, B, KT, P)  # [tau, toff, b, k, i]
    v = np.transpose(v, (0, 4, 3, 1, 2))  # [tau, i, k, toff, b]
    return np.ascontiguousarray(v.reshape(ntt, P, KT * P)).astype(BF16_NP)


_PROG_CACHE: dict = {}


def get_program(t_steps: int = T):
    if t_steps not in _PROG_CACHE:
        _PROG_CACHE[t_steps] = build_program(t_steps)
    return _PROG_CACHE[t_steps]


def kernel(
    input,
    Wih_f,
    Whh_f,
    bih_f,
    bhh_f,
    Wih_b,
    Whh_b,
    bih_b,
    bhh_b,
    W_lin,
    b_lin,
):
    from concourse.bass_utils import run_bass_kernel_spmd

    args = [
        np.asarray(a, dtype=np.float32)
        for a in (Wih_f, Whh_f, bih_f, bhh_f, Wih_b, Whh_b, bih_b, bhh_b, W_lin, b_lin)
    ]
    x = np.asarray(input, dtype=np.float32)
    xT_shared = make_xT(x, T)
    nc = get_program(T)
    in_maps = [make_core_inputs(r, xT_shared, *args) for r in range(N)]
    rr = run_bass_kernel_spmd(nc, in_maps, list(range(N)), **globals().get("RUN_KW", {}))
    res = rr.results
    global LAST_EXEC_NS, LAST_TRACE
    LAST_EXEC_NS = rr.exec_time_ns
    LAST_TRACE = rr.instructions_and_trace
    out = np.concatenate([res[r]["out"] for r in range(N)], axis=1)
    return np.ascontiguousarray(out).astype(np.float32)



# revision 8
# speedup vs baseline: 1.5285x; 1.5285x over previous
"""BiGRU Trainium2 kernel, 8-core SPMD.

Strategy: shard the hidden dimension H=1024 8 ways (128 per core). Each core
computes its 128-wide slice of both GRU directions for the full batch; the
per-step hidden state is exchanged between all cores with SWDGE remote DMA
(SBUF -> SBUF, one receive slot per peer). The recurrence matmul is
hidden-state-stationary: lhsT = h^T tiles (K = H on partitions), rhs = Whh^T
column slices, so the PE streams weight columns at full rate; fwd and bwd
directions run concurrently on the two halves of the PE array (out partition
base 0 / 64).

The input projection xg = x @ Wih^T + biases (both directions) is computed
on-device, interleaved with the recurrence to fill PE idle time, and staged
through DRAM in [t*B + b] row order so each step loads contiguous tiles.

SPMD twist: remote-DMA relative destinations XOR the *physical* NC index and
instruction streams are identical on all cores, so per-core differences live
in data only. Receive slot d on logical core r holds the h-slice of core
sigma_r(d) = FINV[F[r] ^ d] (F = logical->physical NC map); the host permutes
each core's Whh^T / W_lin^T contraction blocks by sigma_r so one static slot
order is correct everywhere.
"""

import os
import sys

sys.path.insert(0, "/opt/trn_rl_repo")

import numpy as np
import ml_dtypes

import concourse.bass as bass
import concourse.mybir as mybir

# ---- problem constants -------------------------------------------------------
B = 64  # batch
T = 512  # sequence length
I = 1024  # input features
H = 1024  # hidden
O = 1024  # output features
N = 8  # cores
KT = 8  # 128-row contraction blocks in H (and I)
P = 128
SL = 128  # per-core H slice
G3 = 3 * SL  # per-core gate columns (r|z|n)

# logical -> physical NC map of this fabric (measured); relative XOR remote-DMA
# routing operates in physical space.
F_MAP = [0, 1, 2, 3, 6, 7, 4, 5]
FINV = [F_MAP.index(i) for i in range(8)]

BF16 = mybir.dt.bfloat16
F32 = mybir.dt.float32
AFT = mybir.ActivationFunctionType


def sigma(r: int, d: int) -> int:
    """H-slice owner whose tile lands in receive slot d on logical core r."""
    return FINV[F_MAP[r] ^ d]


# ---- device program ----------------------------------------------------------


def build_program(t_steps: int = T):
    """One SPMD Bacc program, identical for all 8 cores.

    t_steps must be even; the phase-1 token tiling assumes
    n_tok_tiles = t_steps / 2 (each tile = 2 t-values x 64 batch rows).
    """
    from concourse.bacc import Bacc

    assert t_steps % 2 == 0
    NTT = t_steps // 2  # phase-1 token tiles
    PRO = min(4, NTT)  # tiles processed before step 0
    XPF = 4  # xg prefetch depth (ring)

    DBG = os.environ.get("KDBG", "").split(",")
    no_bcast = "nobcast" in DBG
    no_epi = "noepi" in DBG
    no_rec = "norec" in DBG
    no_gates = "nogates" in DBG
    no_transp = "notransp" in DBG
    no_mmrec = "nommrec" in DBG
    act_only = "actonly" in DBG
    no_tanh = "notanh" in DBG
    no_dvemix = "nodvemix" in DBG

    nc = Bacc(num_swdge_queues=4)
    NQ = 4  # SWDGE queues; queue q's desc-gen runs on Q7 pair q (parallel)
    LEAD = 2  # steps of descriptor pre-generation ahead of the trigger

    # -- IO -------------------------------------------------------------------
    xT = nc.declare_dram_parameter("xT", [NTT, P, KT * P], BF16, isOutput=False)
    wih = nc.declare_dram_parameter("wih", [KT, P, 2 * G3], BF16, isOutput=False)
    whh = nc.declare_dram_parameter("whh", [KT, P, 2 * G3], BF16, isOutput=False)
    wlin = nc.declare_dram_parameter("wlin", [2 * KT, P, SL], BF16, isOutput=False)
    bias1 = nc.declare_dram_parameter("bias1", [1, 2 * G3], BF16, isOutput=False)
    biasn = nc.declare_dram_parameter("biasn", [1, 2 * SL], BF16, isOutput=False)
    blin = nc.declare_dram_parameter("blin", [1, SL], BF16, isOutput=False)
    ident = nc.declare_dram_parameter("ident", [P, P], BF16, isOutput=False)
    ones = nc.declare_dram_parameter("ones", [1, P], BF16, isOutput=False)
    out = nc.declare_dram_parameter("out", [B, SL], F32, isOutput=True)

    # phase-1 output staging through DRAM, [t*64 + b, 384] row order
    xgf_d = nc.dram_tensor("xgf_d", [t_steps * B, G3], BF16)
    xgb_d = nc.dram_tensor("xgb_d", [t_steps * B, G3], BF16)

    n_init_dma = KT + KT + 2 * KT + 5  # whh, wih, wlin blocks + 5 small consts

    def ph1_tile(p: int) -> int:
        """phase-1 processing order: ends inward (0, NTT-1, 1, NTT-2, ...)."""
        return p // 2 if p % 2 == 0 else NTT - 1 - p // 2

    from contextlib import ExitStack

    es = ExitStack()
    with es:
        sem = lambda name: es.enter_context(nc.semaphore(name))
        sbuf = lambda name, shape, dt=BF16: es.enter_context(
            nc.sbuf_tensor(name, shape, dt)
        )
        psum = lambda name, shape, dt: es.enter_context(nc.psum_tensor(name, shape, dt))

        block = es.enter_context(nc.Block())
        init_sem = sem("init_sem")
        hz_sem = sem("hz_sem")
        bar_sem = sem("bar_sem")
        bar_p = sem("bar_p")
        bar_l = sem("bar_l")
        rsem = [[sem(f"rsem{par}_{d}") for d in range(N)] for par in range(2)]
        lsem = [sem("lsem0"), sem("lsem1")]
        prep_q = [sem(f"prep_q{q}") for q in range(NQ)]
        psum_rdy = sem("psum_rdy")
        a2v_r = sem("a2v_r")
        a2v_z = sem("a2v_z")
        a2v_n = sem("a2v_n")
        v2a_np = sem("v2a_np")
        pf_v = sem("pf_v")
        v2p = sem("v2p")
        vch = sem("vch")
        p2v = sem("p2v")
        tdone = sem("tdone")
        xg_dma = [sem(f"xg_dma{i}") for i in range(XPF)]
        xgc_p = sem("xgc_p")
        xt_dma = [sem("xt_dma0"), sem("xt_dma1")]
        p1_rdy = sem("p1_rdy")
        p1_cp = sem("p1_cp")
        p1_w = [sem("p1_w0"), sem("p1_w1")]
        fin_sem = sem("fin_sem")

        whh_s = sbuf("whh_s", [P, KT * 2 * G3])
        wih_s = sbuf("wih_s", [P, KT * 2 * G3])
        wlin_s = sbuf("wlin_s", [P, 2 * KT * SL])
        hbuf = sbuf("hbuf", [P, 2 * N * P])
        xg_s = sbuf("xg_s", [P, XPF * G3])
        xt_s = sbuf("xt_s", [P, 2 * KT * P])
        rz_s = sbuf("rz_s", [P, 2 * SL])
        t1_s = sbuf("t1_s", [P, SL])
        npre_s = sbuf("npre_s", [P, SL])
        n_s = sbuf("n_s", [P, SL])
        s1_s = sbuf("s1_s", [P, SL])
        s2_s = sbuf("s2_s", [P, SL])
        hst_s = sbuf("hst_s", [P, SL])
        hgn_s = sbuf("hgn_s", [P, SL])
        tb_s = sbuf("tb_s", [P, 2 * P])
        xgf_st = sbuf("xgf_st", [P, 2 * G3])
        xgb_st = sbuf("xgb_st", [P, 2 * G3])
        ident_s = sbuf("ident_s", [P, P])
        ones_s = sbuf("ones_s", [1, P])
        bias1_s = sbuf("bias1_s", [1, 2 * G3])
        biasn_s = sbuf("biasn_s", [1, 2 * SL])
        blin_s = sbuf("blin_s", [1, SL])
        out_s = sbuf("out_s", [B, SL], F32)
        # separate tensors so double-buffers land in different PSUM banks
        # (PE-write + DVE-read of one bank is a hardware fault)
        ps_rec0 = psum("ps_rec0", [P, G3], F32)
        ps_rec1 = psum("ps_rec1", [P, G3], F32)
        ps_t0 = psum("ps_t0", [P, P], BF16)
        ps_t1 = psum("ps_t1", [P, P], BF16)
        ps_p1f = psum("ps_p1f", [P, G3], F32)
        ps_p1b = psum("ps_p1b", [P, G3], F32)
        ps_rec = [ps_rec0, ps_rec1]
        ps_t = [ps_t0, ps_t1]

        def hb(t):
            """hbuf column offset of the buffer read at step t."""
            return (t % 2) * N * P

        # ---------------- SYNC: all HWDGE DMA traffic ---------------------
        @block.sync
        def _(s):
            for k in range(KT):
                s.dma_start(
                    out=whh_s[:, k * 2 * G3 : (k + 1) * 2 * G3], in_=whh[k, :, :]
                ).then_inc(init_sem, 16)
                s.dma_start(
                    out=wih_s[:, k * 2 * G3 : (k + 1) * 2 * G3], in_=wih[k, :, :]
                ).then_inc(init_sem, 16)
            for k in range(2 * KT):
                s.dma_start(
                    out=wlin_s[:, k * SL : (k + 1) * SL], in_=wlin[k, :, :]
                ).then_inc(init_sem, 16)
            s.dma_start(out=ident_s[:, :], in_=ident[:, :]).then_inc(init_sem, 16)
            s.dma_start(out=ones_s[:, :], in_=ones[:, :]).then_inc(init_sem, 16)
            s.dma_start(out=bias1_s[:, :], in_=bias1[:, :]).then_inc(init_sem, 16)
            s.dma_start(out=biasn_s[:, :], in_=biasn[:, :]).then_inc(init_sem, 16)
            s.dma_start(out=blin_s[:, :], in_=blin[:, :]).then_inc(init_sem, 16)

            def load_xt(p):
                if p >= NTT:
                    return
                if p >= 2:
                    s.wait_ge(p1_rdy, p - 1)  # xt ring slot free
                s.dma_start(
                    out=xt_s[:, (p % 2) * KT * P : ((p % 2) + 1) * KT * P],
                    in_=xT[ph1_tile(p), :, :],
                ).then_inc(xt_dma[p % 2], 16)

            def write_ph1(p):
                if p >= NTT:
                    return
                tau = ph1_tile(p)
                s.wait_ge(p1_cp, 2 * (p + 1))
                s.dma_start(
                    out=xgf_d[2 * tau * B : 2 * tau * B + P, :],
                    in_=xgf_st[:, (p % 2) * G3 : (p % 2) * G3 + G3],
                ).then_inc(p1_w[p % 2], 16)
                s.dma_start(
                    out=xgb_d[2 * tau * B : 2 * tau * B + P, :],
                    in_=xgb_st[:, (p % 2) * G3 : (p % 2) * G3 + G3],
                ).then_inc(p1_w[p % 2], 16)

            def load_xg(t):
                if no_rec or t >= t_steps or t < 0:
                    return
                if load_xg.done >= t + 1:
                    return
                load_xg.done = t + 1
                # phase-1 tiles 0..need_p-1 cover fwd row t and bwd row T-1-t
                need_p = min(2 * (t // 2) + 2, NTT)
                s.wait_ge(p1_w[0], 32 * (need_p - need_p // 2))
                s.wait_ge(p1_w[1], 32 * (need_p // 2))
                if t >= XPF:
                    s.wait_ge(v2a_np, t - XPF + 1)
                    s.wait_ge(xgc_p, t - XPF + 1)
                slot = (t % XPF) * G3
                s.dma_start(
                    out=xg_s[0:B, slot : slot + G3],
                    in_=xgf_d[t * B : (t + 1) * B, :],
                ).then_inc(xg_dma[t % XPF], 16)
                s.dma_start(
                    out=xg_s[B:P, slot : slot + G3],
                    in_=xgb_d[(t_steps - 1 - t) * B : (t_steps - t) * B, :],
                ).then_inc(xg_dma[t % XPF], 16)

            # prologue: interleave so no FIFO head-of-line cycle forms
            # (load_xt(p+2) transitively needs write_ph1(p-2) through PE/DVE)
            for p in range(4):
                load_xt(p)
            write_ph1(0)
            load_xt(4)
            write_ph1(1)
            load_xt(5)
            write_ph1(2)
            write_ph1(3)
            load_xg.done = 0
            for t in range(XPF):
                load_xg(t)
            for t in range(t_steps):
                write_ph1(PRO + t)
                load_xt(PRO + t + 2)
                load_xg(t + XPF - 1)

            s.wait_ge(fin_sem, 1)
            s.dma_start(out=out[:, :], in_=out_s[:, :]).then_inc(fin_sem, 16)

        # ---------------- PE: matmuls, transpose, phase-1 ------------------
        @block.tensor
        def _(pe):
            def ph1_work(p):
                if p >= NTT:
                    return
                pe.wait_ge(xt_dma[p % 2], 16 * (p // 2 + 1))
                if p >= 1:
                    pe.wait_ge(p1_cp, 2 * p)  # psum consumed by DVE copies
                xo = (p % 2) * KT * P
                for k in range(KT):
                    lt = xt_s[:, xo + k * P : xo + (k + 1) * P]
                    pe.matmul(
                        ps_p1f[:, :],
                        lt,
                        wih_s[:, k * 2 * G3 : k * 2 * G3 + G3],
                        start=(k == 0),
                        stop=False,
                    )
                    pe.matmul(
                        ps_p1b[:, :],
                        lt,
                        wih_s[:, k * 2 * G3 + G3 : (k + 1) * 2 * G3],
                        start=(k == 0),
                        stop=False,
                    )
                pe.matmul(
                    ps_p1f[:, :],
                    ones_s[0:1, :],
                    bias1_s[0:1, 0:G3],
                    start=False,
                    stop=True,
                )
                pe.matmul(
                    ps_p1b[:, :],
                    ones_s[0:1, :],
                    bias1_s[0:1, G3 : 2 * G3],
                    start=False,
                    stop=True,
                ).then_inc(p1_rdy, 1)

            pe.wait_ge(init_sem, 16 * n_init_dma)
            pe.wait_ge(hz_sem, 2)
            for p in range(PRO):
                ph1_work(p)

            for t in range(t_steps):
                ps = ps_rec[t % 2]
                if no_rec:
                    ph1_work(PRO + t)
                    continue
                pass  # per-slot rsem waits moved into the d-loop below
                if t >= 2:
                    pe.wait_ge(a2v_z, 2 * (t - 1))
                    pe.wait_ge(pf_v, t - 1)
                pe.wait_ge(xg_dma[t % XPF], 32 * (t // XPF + 1))
                hbo = hb(t)
                slot = (t % XPF) * G3
                if no_mmrec:
                    pe.matmul(
                        ps[:, 0 : 2 * SL],
                        ident_s[:, :],
                        xg_s[:, slot : slot + 2 * SL],
                        start=True,
                        stop=True,
                    ).then_inc(xgc_p, 1)
                    pe.matmul(
                        ps[0:B, 2 * SL : G3],
                        ones_s[0:1, 0:B],
                        biasn_s[0:1, 0:SL],
                        start=True,
                        stop=True,
                    )
                    pe.matmul(
                        ps[B:P, 2 * SL : G3],
                        ones_s[0:1, B:P],
                        biasn_s[0:1, SL : 2 * SL],
                        start=True,
                        stop=True,
                        skip_group_check=True,
                    ).then_inc(psum_rdy, 1)
                if not no_mmrec:
                    # d=0 opens the accumulation (start marks the whole bank
                    # pending-zero per partition half); d=7 closes it; the xg and
                    # bhh_n matmuls then accumulate on top (group-check skipped:
                    # their region is a slice of the already-opened groups).
                    for d in range(N):
                        if t >= 1 and not no_bcast:
                            # per-slot wait: start contracting as tiles arrive
                            pe.wait_ge(rsem[(t - 1) % 2][d], 2 * ((t - 1) // 2 + 1))
                        pe.matmul(
                            ps[0:B, :],
                            hbuf[:, hbo + d * P : hbo + d * P + B],
                            whh_s[:, d * 2 * G3 : d * 2 * G3 + G3],
                            start=(d == 0),
                            stop=(d == N - 1),
                        )
                        pe.matmul(
                            ps[B:P, :],
                            hbuf[:, hbo + d * P + B : hbo + (d + 1) * P],
                            whh_s[:, d * 2 * G3 + G3 : (d + 1) * 2 * G3],
                            start=(d == 0),
                            stop=(d == N - 1),
                            skip_group_check=True,
                        )
                    pe.matmul(
                        ps[:, 0 : 2 * SL],
                        ident_s[:, :],
                        xg_s[:, slot : slot + 2 * SL],
                        start=False,
                        stop=False,
                        skip_group_check=True,
                    ).then_inc(xgc_p, 1)
                    pe.matmul(
                        ps[0:B, 2 * SL : G3],
                        ones_s[0:1, 0:B],
                        biasn_s[0:1, 0:SL],
                        start=False,
                        stop=False,
                        skip_group_check=True,
                    )
                    pe.matmul(
                        ps[B:P, 2 * SL : G3],
                        ones_s[0:1, B:P],
                        biasn_s[0:1, SL : 2 * SL],
                        start=False,
                        stop=False,
                        skip_group_check=True,
                    ).then_inc(psum_rdy, 1)

                # phase-1 fill while the gates run on ACT/DVE
                ph1_work(PRO + t)

                # transpose h_new into the broadcast source layout
                if not no_transp:
                    pe.wait_ge(v2p, t + 1)
                    pe.transpose(ps_t[t % 2][:, :], hst_s[:, :], ident_s[:, :]).then_inc(
                        p2v, 1
                    )

            if no_epi or no_rec:
                pe.wait_ge(p1_cp, 2 * NTT)
                pe.matmul(
                    ps_p1f[0:B, 0:SL],
                    ident_s[:, 0:B],
                    blin_s[0:1, :] if False else wlin_s[0:128, 0:SL],
                    start=True,
                    stop=True,
                ).then_inc(psum_rdy, 1 if no_rec else t_steps + 1)
                return

            # final linear: out = [h_fwd | h_bwd] @ W_lin^T + b_lin
            if not no_bcast:
                for d in range(N):
                    pe.wait_ge(
                        rsem[(t_steps - 1) % 2][d], 2 * ((t_steps - 1) // 2 + 1)
                    )
            pe.wait_ge(p1_cp, 2 * NTT)  # ps_p1f free
            hbo = hb(t_steps)
            for d in range(N):
                pe.matmul(
                    ps_p1f[0:B, 0:SL],
                    hbuf[:, hbo + d * P : hbo + d * P + B],
                    wlin_s[:, d * SL : (d + 1) * SL],
                    start=(d == 0),
                    stop=False,
                )
            for d in range(N):
                pe.matmul(
                    ps_p1f[0:B, 0:SL],
                    hbuf[:, hbo + d * P + B : hbo + (d + 1) * P],
                    wlin_s[:, (N + d) * SL : (N + d + 1) * SL],
                    start=False,
                    stop=False,
                )
            pe.matmul(
                ps_p1f[0:B, 0:SL],
                ones_s[0:1, 0:B],
                blin_s[0:1, :],
                start=False,
                stop=True,
            ).then_inc(psum_rdy, 1)

        # ---------------- ACT: sigmoids + tanh ----------------------------
        @block.scalar
        def _(a):
            if no_rec or no_gates:
                return
            for t in range(t_steps):
                ps = ps_rec[t % 2]
                a.wait_ge(psum_rdy, t + 1)
                a.activation(rz_s[:, 0:SL], ps[:, 0:SL], AFT.Sigmoid).then_inc(
                    a2v_r, 1
                )
                a.activation(
                    rz_s[:, SL : 2 * SL], ps[:, SL : 2 * SL], AFT.Sigmoid
                ).then_inc(a2v_z, 1)
                a.activation(hgn_s[:, :], ps[:, 2 * SL : G3], AFT.Copy).then_inc(
                    a2v_z, 1
                )
                if act_only or no_tanh:
                    a.activation(n_s[:, :], npre_s[:, :], AFT.Sigmoid).then_inc(
                        a2v_n, 1
                    )
                else:
                    a.wait_ge(v2a_np, t + 1)
                    a.activation(n_s[:, :], npre_s[:, :], AFT.Tanh).then_inc(a2v_n, 1)

        # ---------------- DVE: gate arithmetic, copies --------------------
        @block.vector
        def _(v):
            v.memset(hbuf[:, :], 0.0).then_inc(hz_sem, 1)
            v.memset(hst_s[:, :], 0.0).then_inc(hz_sem, 1)
            v.wait_ge(hz_sem, 2)

            def ph1_copy(p):
                if p >= NTT:
                    return
                v.wait_ge(p1_rdy, p + 1)
                if p >= 2:
                    v.wait_ge(p1_w[p % 2], 32 * (p // 2))  # staging slot free
                v.tensor_copy(
                    xgf_st[:, (p % 2) * G3 : (p % 2) * G3 + G3], ps_p1f[:, :]
                )
                v.tensor_copy(
                    xgb_st[:, (p % 2) * G3 : (p % 2) * G3 + G3], ps_p1b[:, :]
                ).then_inc(p1_cp, 2)

            for p in range(PRO):
                ph1_copy(p)

            for t in range(t_steps):
                if no_rec:
                    ph1_copy(PRO + t)
                    continue
                ps = ps_rec[t % 2]
                slot = (t % XPF) * G3
                if no_gates:
                    v.wait_ge(psum_rdy, t + 1)
                    v.tensor_copy(s1_s[:, :], hst_s[:, :]).then_inc(v2p, 1)
                    if not no_transp:
                        v.wait_ge(p2v, t + 1)
                        v.tensor_copy(
                            tb_s[:, (t % 2) * P : (t % 2) * P + P], ps_t[t % 2][:, :]
                        ).then_inc(tdone, 1)
                    ph1_copy(PRO + t)
                    continue
                v.wait_ge(a2v_r, t + 1)
                if act_only:
                    v.tensor_copy(s1_s[:, :], hst_s[:, :]).then_inc(v2p, 1)
                    if not no_transp:
                        v.wait_ge(p2v, t + 1)
                        v.tensor_copy(
                            tb_s[:, (t % 2) * P : (t % 2) * P + P], ps_t[t % 2][:, :]
                        ).then_inc(tdone, 1)
                    ph1_copy(PRO + t)
                    continue
                # t1 = r * hg_n (hg_n staged through SBUF by ACT: a DVE
                # TensorTensor read of PSUM hard-faults this device)
                v.wait_ge(a2v_z, 2 * t + 2)
                v.tensor_mul(t1_s[:, :], rz_s[:, 0:SL], hgn_s[:, :]).then_inc(
                    pf_v, 1
                )
                # n_pre = t1 + xg_n
                v.wait_ge(pf_v, t + 1)  # t1 writeback drained
                v.wait_ge(xg_dma[t % XPF], 32 * (t // XPF + 1))
                v.tensor_add(
                    npre_s[:, :], t1_s[:, :], xg_s[:, slot + 2 * SL : slot + G3]
                ).then_inc(v2a_np, 1)
                v.wait_ge(a2v_n, t + 1)
                # h_new = n + z*(h - n)
                if t >= 1:
                    v.wait_ge(v2p, t)  # prior h_new writeback drained
                v.tensor_sub(s1_s[:, :], hst_s[:, :], n_s[:, :]).then_inc(vch, 1)
                v.wait_ge(a2v_z, 2 * t + 1)
                v.wait_ge(vch, 2 * t + 1)
                v.tensor_mul(s2_s[:, :], rz_s[:, SL : 2 * SL], s1_s[:, :]).then_inc(
                    vch, 1
                )
                v.wait_ge(vch, 2 * t + 2)
                v.tensor_add(hst_s[:, :], n_s[:, :], s2_s[:, :]).then_inc(v2p, 1)

                # move the transposed tile into the broadcast source buffer
                if not no_transp:
                    v.wait_ge(p2v, t + 1)
                    if t >= 2 and not no_bcast:
                        v.wait_ge(lsem[t % 2], 128 * (t // 2))  # t-2 sends done
                    v.tensor_copy(
                        tb_s[:, (t % 2) * P : (t % 2) * P + P], ps_t[t % 2][:, :]
                    ).then_inc(tdone, 1)

                ph1_copy(PRO + t)

            v.wait_ge(psum_rdy, 1 if no_rec else t_steps + 1)
            v.tensor_copy(out_s[:, :], ps_p1f[0:B, 0:SL]).then_inc(fin_sem, 1)

        # ---------------- GPSIMD: remote broadcasts ------------------------
        @block.gpsimd
        def _(g):
            if no_bcast or no_rec:
                return
            # start barrier: no core may broadcast into peers' hbuf until every
            # core has zero-initialized its own hbuf.
            g.wait_ge(hz_sem, 1)
            g.remote_sem_update_broadcast(
                remote_sem=bar_sem,
                local_sem=bar_l,
                rdests=[(0, k) for k in range(N)],
            ).then_inc(bar_p, 1)
            g.wait_ge(bar_p, 1)
            g.trigger_dma(count=1)
            g.wait_ge(bar_sem, 16)
            # barrier passed: every core has zeroed hbuf, so peers' step-0
            # broadcasts may now arrive at any time.

            # Descriptor generation is decoupled from triggering: descs only
            # embed addresses (tb_s / hbuf slots repeat with period 2), so they
            # can be generated LEAD steps early, spread over 4 SWDGE queues
            # (one Q7 pair each). Only the trigger must respect ordering: it
            # fires after this step's matmuls (psum_rdy, the transitive
            # receiver-read chain) and the tb_s copy (tdone).
            def descgen(s):
                if s >= t_steps:
                    return
                for d in range(N):
                    rd = [None] * N
                    rd[d] = (0, d)
                    g.remote_dma_broadcast(
                        out_ap=hbuf[:, hb(s + 1) + d * P : hb(s + 1) + (d + 1) * P],
                        in_ap=tb_s[:, (s % 2) * P : (s % 2) * P + P],
                        remote_sem=rsem[s % 2][d],
                        local_sem=lsem[s % 2],
                        rdests=rd,
                        queue_num=d % NQ,
                    ).then_inc(prep_q[d % NQ], 1)

            for s in range(LEAD):
                descgen(s)
            for t in range(t_steps):
                descgen(t + LEAD)
                g.wait_ge(psum_rdy, t + 1)
                g.wait_ge(tdone, t + 1)
                for q in range(NQ):
                    g.wait_ge(prep_q[q], (N // NQ) * (t + 1))
                    g.trigger_dma(count=N // NQ, queue_num=q)

    nc.finalize()
    return nc


# ---- host-side input preparation ---------------------------------------------

BF16_NP = ml_dtypes.bfloat16


def _own_rows(r: int) -> np.ndarray:
    """Row indices (into 3H) of core r's r/z/n gate slices."""
    base = np.arange(r * SL, (r + 1) * SL)
    return np.concatenate([base, H + base, 2 * H + base])


def make_core_inputs(
    r,
    xT_shared,
    Wih_f,
    Whh_f,
    bih_f,
    bhh_f,
    Wih_b,
    Whh_b,
    bih_b,
    bhh_b,
    W_lin,
    b_lin,
):
    rows = _own_rows(r)
    perm = [sigma(r, d) for d in range(N)]

    def wih_pack():
        wf = np.ascontiguousarray(Wih_f[rows, :].T)  # [I, 384]
        wb = np.ascontiguousarray(Wih_b[rows, :].T)
        o = np.empty((KT, P, 2 * G3), dtype=BF16_NP)
        for k in range(KT):
            o[k, :, 0:G3] = wf[k * P : (k + 1) * P, :]
            o[k, :, G3 : 2 * G3] = wb[k * P : (k + 1) * P, :]
        return o

    def whh_pack():
        wf = np.ascontiguousarray(Whh_f[rows, :].T)  # [H, 384]
        wb = np.ascontiguousarray(Whh_b[rows, :].T)
        o = np.empty((KT, P, 2 * G3), dtype=BF16_NP)
        for d in range(N):
            s = perm[d]
            o[d, :, 0:G3] = wf[s * P : (s + 1) * P, :]
            o[d, :, G3 : 2 * G3] = wb[s * P : (s + 1) * P, :]
        return o

    def wlin_pack():
        wl = np.ascontiguousarray(W_lin[r * SL : (r + 1) * SL, :].T)  # [2H, 128]
        o = np.empty((2 * KT, P, SL), dtype=BF16_NP)
        for d in range(N):
            s = perm[d]
            o[d] = wl[s * P : (s + 1) * P, :]
            o[N + d] = wl[H + s * P : H + (s + 1) * P, :]
        return o

    brz_f = (bih_f + bhh_f)[rows]
    brz_b = (bih_b + bhh_b)[rows]
    b1 = np.empty((1, 2 * G3), dtype=BF16_NP)
    b1[0, 0 : 2 * SL] = brz_f[0 : 2 * SL]
    b1[0, 2 * SL : G3] = bih_f[rows][2 * SL : G3]
    b1[0, G3 : G3 + 2 * SL] = brz_b[0 : 2 * SL]
    b1[0, G3 + 2 * SL : 2 * G3] = bih_b[rows][2 * SL : G3]

    bn = np.empty((1, 2 * SL), dtype=BF16_NP)
    bn[0, 0:SL] = bhh_f[rows][2 * SL : G3]
    bn[0, SL : 2 * SL] = bhh_b[rows][2 * SL : G3]

    return {
        "xT": xT_shared,
        "wih": wih_pack(),
        "whh": whh_pack(),
        "wlin": wlin_pack(),
        "bias1": b1,
        "biasn": bn,
        "blin": b_lin[r * SL : (r + 1) * SL].reshape(1, SL).astype(BF16_NP),
        "ident": np.eye(P, dtype=BF16_NP),
        "ones": np.ones((1, P), dtype=BF16_NP),
    }


def make_xT(input_btI: np.ndarray, t_steps: int = T) -> np.ndarray:
    """[B,T,I] -> [NTT, P, KT*P] bf16, token order (k, t_off, b) in the free dim."""
    ntt = t_steps // 2
    xt = np.transpose(input_btI, (1, 0, 2))  # [T, B, I]
    v = xt.reshape(ntt, 2, B, KT, P)  # [tau, toff, b, k, i]
    v = np.transpose(v, (0, 4, 3, 1, 2))  # [tau, i, k, toff, b]
    return np.ascontiguousarray(v.reshape(ntt, P, KT * P)).astype(BF16_NP)


_PROG_CACHE: dict = {}


def get_program(t_steps: int = T):
    if t_steps not in _PROG_CACHE:
        _PROG_CACHE[t_steps] = build_program(t_steps)
    return _PROG_CACHE[t_steps]


def kernel(
    input,
    Wih_f,
    Whh_f,
    bih_f,
    bhh_f,
    Wih_b,
    Whh_b,
    bih_b,
    bhh_b,
    W_lin,
    b_lin,
):
    from concourse.bass_utils import run_bass_kernel_spmd

    args = [
        np.asarray(a, dtype=np.float32)
        for a in (Wih_f, Whh_f, bih_f, bhh_f, Wih_b, Whh_b, bih_b, bhh_b, W_lin, b_lin)
    ]
    x = np.asarray(input, dtype=np.float32)
    xT_shared = make_xT(x, T)
    nc = get_program(T)
    in_maps = [make_core_inputs(r, xT_shared, *args) for r in range(N)]
    rr = run_bass_kernel_spmd(nc, in_maps, list(range(N)), **globals().get("RUN_KW", {}))
    res = rr.results
    global LAST_EXEC_NS, LAST_TRACE
    LAST_EXEC_NS = rr.exec_time_ns
    LAST_TRACE = rr.instructions_and_trace
    out = np.concatenate([res[r]["out"] for r in range(N)], axis=1)
    return np.ascontiguousarray(out).astype(np.float32)



# revision 9
# speedup vs baseline: 2.7211x; 1.7802x over previous
"""BiGRU Trainium2 kernel, 8-core SPMD, direction-split (4+4).

Cores 0-3 run the FWD GRU, cores 4-7 the BWD GRU (physical quads, same-die).
Each core owns a 256-wide h-slice of its direction, held as two 128-wide
subslices stacked on PSUM/SBUF partition halves (64 batch x 2 subs = 128
partitions) -- structurally identical tiles to the mixed baseline, so the
ACT/DVE gate pipeline is unchanged. Per step each core broadcasts its
transposed h tile [128 h x (64b sub0 | 64b sub1)] to its 3 quad peers + self
with FOUR relative SWDGE broadcast calls, one per SWDGE queue (one Q7 pair
each, descriptor generation pre-run LEAD steps ahead; only the trigger sits
in the step's critical path).

Phase-1 (xg = x @ Wih^T + bias) is uniform: one half-tile (384 gate cols of a
2-token tile) per step for all 512 steps; fwd cores consume x tiles in
forward order, bwd cores get a host-reversed copy of x, so the device
program is identical on all cores.

Final linear: each core computes out^T partials [128 o x 64 b] for its own
O-slice and its cross-pair's O-slice over its direction's h; the cross
partial is sent to the paired core on the other die (relative slot 6) and
added there.
"""

import os
import sys

sys.path.insert(0, "/opt/trn_rl_repo")

import numpy as np
import ml_dtypes

import concourse.bass as bass
import concourse.mybir as mybir

B = 64
T = 512
I = 1024
H = 1024
O = 1024
N = 8
NS = 4  # broadcast slots (quad size)
KT = 8  # 128-row contraction blocks in own-dir H (and I)
P = 128
SL = 128
G3 = 3 * SL  # gate cols per partition-half (r|z|n of one 128-subslice)

F_MAP = [0, 1, 2, 3, 6, 7, 4, 5]
FINV = [F_MAP.index(i) for i in range(8)]

BF16 = mybir.dt.bfloat16
F32 = mybir.dt.float32
AFT = mybir.ActivationFunctionType


def sigma_in(r: int, d: int) -> int:
    """In-quad index of the sender whose tile lands in slot d on core r."""
    return FINV[F_MAP[r] ^ d] % 4


def build_program(t_steps: int = T):
    from concourse.bacc import Bacc

    assert t_steps % 2 == 0
    NTT = t_steps // 2
    NH = t_steps  # ph1 half-tiles (one per step)
    LP = 6  # ph1 half-index lead over the step loop
    XPF = 4  # xg prefetch ring depth
    LEAD = 2  # broadcast descriptor pre-generation lead (steps)
    NQ = 4

    nc = Bacc(num_swdge_queues=NQ)

    xT = nc.declare_dram_parameter("xT", [NTT, P, KT * P], BF16, isOutput=False)
    wih = nc.declare_dram_parameter("wih", [KT, P, 2 * G3], BF16, isOutput=False)
    whh = nc.declare_dram_parameter("whh", [KT, P, 2 * G3], BF16, isOutput=False)
    wlin = nc.declare_dram_parameter("wlin", [2 * KT, P, SL], BF16, isOutput=False)
    bias1 = nc.declare_dram_parameter("bias1", [1, 2 * G3], BF16, isOutput=False)
    biasn = nc.declare_dram_parameter("biasn", [1, 2 * SL], BF16, isOutput=False)
    blin = nc.declare_dram_parameter("blin", [1, SL], BF16, isOutput=False)
    ident = nc.declare_dram_parameter("ident", [P, P], BF16, isOutput=False)
    ones = nc.declare_dram_parameter("ones", [1, P], BF16, isOutput=False)
    out = nc.declare_dram_parameter("out", [SL, B], F32, isOutput=True)

    xg_d = nc.dram_tensor("xg_d", [t_steps * B, 2 * G3], BF16)

    n_init_dma = KT + KT + 2 * KT + 5

    from contextlib import ExitStack

    es = ExitStack()
    with es:
        sem = lambda name: es.enter_context(nc.semaphore(name))
        sbuf = lambda name, shape, dt=BF16: es.enter_context(
            nc.sbuf_tensor(name, shape, dt)
        )
        psum = lambda name, shape, dt: es.enter_context(nc.psum_tensor(name, shape, dt))

        block = es.enter_context(nc.Block())
        init_sem = sem("init_sem")
        hz_sem = sem("hz_sem")
        bar_sem = sem("bar_sem")
        bar_p = sem("bar_p")
        bar_l = sem("bar_l")
        rsem = [[sem(f"rsem{par}_{d}") for d in range(NS)] for par in range(2)]
        lsem = [sem(f"lsem{q}") for q in range(NQ)]
        prep_q = [sem(f"prep_q{q}") for q in range(NQ)]
        psum_rdy = sem("psum_rdy")
        a2v_r = sem("a2v_r")
        a2v_z = sem("a2v_z")
        a2v_n = sem("a2v_n")
        v2a_np = sem("v2a_np")
        pf_v = sem("pf_v")
        v2p = sem("v2p")
        vch = sem("vch")
        p2v = sem("p2v")
        tdone = sem("tdone")
        xg_dma = [sem(f"xg_dma{i}") for i in range(XPF)]
        xgc_p = sem("xgc_p")
        xt_dma = [sem("xt_dma0"), sem("xt_dma1")]
        p1_rdy = sem("p1_rdy")
        p1_cp = sem("p1_cp")
        p1_w = sem("p1_w")
        lin_rdy = sem("lin_rdy")
        xch_s = sem("xch_s")
        rsem_x = sem("rsem_x")
        lsem_x = sem("lsem_x")
        prep_x = sem("prep_x")
        fin_sem = sem("fin_sem")

        whh_s = sbuf("whh_s", [P, KT * 2 * G3])
        wih_s = sbuf("wih_s", [P, KT * 2 * G3])
        wlin_s = sbuf("wlin_s", [P, 2 * KT * SL])
        hbuf = sbuf("hbuf", [P, 2 * NS * P])
        xg_s = sbuf("xg_s", [P, XPF * G3])
        xt_s = sbuf("xt_s", [P, 2 * KT * P])
        rz_s = sbuf("rz_s", [P, 2 * SL])
        t1_s = sbuf("t1_s", [P, SL])
        npre_s = sbuf("npre_s", [P, SL])
        n_s = sbuf("n_s", [P, SL])
        s1_s = sbuf("s1_s", [P, SL])
        s2_s = sbuf("s2_s", [P, SL])
        hst_s = sbuf("hst_s", [P, SL])
        hgn_s = sbuf("hgn_s", [P, SL])
        tb_s = sbuf("tb_s", [P, 2 * P])
        xst = sbuf("xst", [P, 2 * G3])
        ident_s = sbuf("ident_s", [P, P])
        ones_s = sbuf("ones_s", [1, P])
        bias1_s = sbuf("bias1_s", [1, 2 * G3])
        biasn_s = sbuf("biasn_s", [1, 2 * SL])
        blin_s = sbuf("blin_s", [1, SL])
        lino_s = sbuf("lino_s", [P, B], F32)
        linx_s = sbuf("linx_s", [P, B], F32)
        linr_s = sbuf("linr_s", [P, B], F32)
        out_s = sbuf("out_s", [P, B], F32)
        ps_rec0 = psum("ps_rec0", [P, G3], F32)
        ps_rec1 = psum("ps_rec1", [P, G3], F32)
        ps_t0 = psum("ps_t0", [P, P], BF16)
        ps_t1 = psum("ps_t1", [P, P], BF16)
        ps_p10 = psum("ps_p10", [P, G3], F32)
        ps_p11 = psum("ps_p11", [P, G3], F32)
        ps_lo = psum("ps_lo", [P, B], F32)
        ps_lx = psum("ps_lx", [P, B], F32)
        ps_rec = [ps_rec0, ps_rec1]
        ps_t = [ps_t0, ps_t1]
        ps_p1 = [ps_p10, ps_p11]

        def hb(t):
            return (t % 2) * NS * P

        # ---------------- SYNC: HWDGE DMA ---------------------------------
        @block.sync
        def _(s):
            for k in range(KT):
                s.dma_start(
                    out=whh_s[:, k * 2 * G3 : (k + 1) * 2 * G3], in_=whh[k, :, :]
                ).then_inc(init_sem, 16)
                s.dma_start(
                    out=wih_s[:, k * 2 * G3 : (k + 1) * 2 * G3], in_=wih[k, :, :]
                ).then_inc(init_sem, 16)
            for k in range(2 * KT):
                s.dma_start(
                    out=wlin_s[:, k * SL : (k + 1) * SL], in_=wlin[k, :, :]
                ).then_inc(init_sem, 16)
            s.dma_start(out=ident_s[:, :], in_=ident[:, :]).then_inc(init_sem, 16)
            s.dma_start(out=ones_s[:, :], in_=ones[:, :]).then_inc(init_sem, 16)
            s.dma_start(out=bias1_s[:, :], in_=bias1[:, :]).then_inc(init_sem, 16)
            s.dma_start(out=biasn_s[:, :], in_=biasn[:, :]).then_inc(init_sem, 16)
            s.dma_start(out=blin_s[:, :], in_=blin[:, :]).then_inc(init_sem, 16)

            def load_xt(p):
                if p >= NTT or p < 0:
                    return
                if load_xt.done >= p + 1:
                    return
                load_xt.done = p + 1
                if p >= 2:
                    s.wait_ge(p1_rdy, 2 * p - 2)  # tile p-2 fully consumed
                s.dma_start(
                    out=xt_s[:, (p % 2) * KT * P : ((p % 2) + 1) * KT * P],
                    in_=xT[p, :, :],
                ).then_inc(xt_dma[p % 2], 16)

            load_xt.done = 0

            def write_ph1(h):
                if h >= NH:
                    return
                s.wait_ge(p1_cp, h + 1)
                p, c = h // 2, h % 2
                s.dma_start(
                    out=xg_d[2 * p * B : 2 * p * B + P, c * G3 : (c + 1) * G3],
                    in_=xst[:, c * G3 : (c + 1) * G3],
                ).then_inc(p1_w, 16)

            def load_xg(t):
                if t >= t_steps or t < 0:
                    return
                if load_xg.done >= t + 1:
                    return
                load_xg.done = t + 1
                s.wait_ge(p1_w, 16 * min(t + 2, NH))
                if t >= XPF:
                    s.wait_ge(v2a_np, t - XPF + 1)
                    s.wait_ge(xgc_p, t - XPF + 1)
                slot = (t % XPF) * G3
                s.dma_start(
                    out=xg_s[0:B, slot : slot + G3],
                    in_=xg_d[t * B : (t + 1) * B, 0:G3],
                ).then_inc(xg_dma[t % XPF], 16)
                s.dma_start(
                    out=xg_s[B:P, slot : slot + G3],
                    in_=xg_d[t * B : (t + 1) * B, G3 : 2 * G3],
                ).then_inc(xg_dma[t % XPF], 16)

            load_xg.done = 0

            # prologue: tiles 0..3, ph1 writes 0..LP-1, xg prefetch 0..XPF-1
            load_xt(0)
            load_xt(1)
            write_ph1(0)
            load_xt(2)
            write_ph1(1)
            load_xg(0)
            load_xt(3)
            write_ph1(2)
            write_ph1(3)
            load_xg(1)
            write_ph1(4)
            load_xg(2)
            write_ph1(5)
            load_xg(3)

            for t in range(t_steps):
                write_ph1(t + LP)
                if (t + LP) % 2 == 0:
                    load_xt((t + LP) // 2 + 1)
                load_xg(t + XPF - 1)

            s.wait_ge(fin_sem, 1)
            s.dma_start(out=out[:, :], in_=out_s[:, :]).then_inc(fin_sem, 16)

        # ---------------- PE ------------------------------------------------
        @block.tensor
        def _(pe):
            def ph1_half(h):
                if h >= NH:
                    return
                p, c = h // 2, h % 2
                pe.wait_ge(xt_dma[p % 2], 16 * (p // 2 + 1))
                if h >= 2:
                    pe.wait_ge(p1_cp, h - 1)  # psum h-2 copied out
                ps = ps_p1[h % 2]
                xo = (p % 2) * KT * P
                for k in range(KT):
                    pe.matmul(
                        ps[:, :],
                        xt_s[:, xo + k * P : xo + (k + 1) * P],
                        wih_s[:, k * 2 * G3 + c * G3 : k * 2 * G3 + (c + 1) * G3],
                        start=(k == 0),
                        stop=False,
                    )
                pe.matmul(
                    ps[:, :],
                    ones_s[0:1, :],
                    bias1_s[0:1, c * G3 : (c + 1) * G3],
                    start=False,
                    stop=True,
                ).then_inc(p1_rdy, 1)

            pe.wait_ge(init_sem, 16 * n_init_dma)
            pe.wait_ge(hz_sem, 2)
            for h in range(LP):
                ph1_half(h)

            for t in range(t_steps):
                ps = ps_rec[t % 2]
                if t >= 2:
                    pe.wait_ge(a2v_z, 2 * (t - 1))
                    pe.wait_ge(pf_v, t - 1)
                hbo = hb(t)
                for kb in range(2 * NS):
                    d = kb // 2
                    if t >= 1 and kb % 2 == 0:
                        pe.wait_ge(rsem[(t - 1) % 2][d], 2 * ((t - 1) // 2 + 1))
                    lt = hbuf[:, hbo + kb * B : hbo + (kb + 1) * B]
                    pe.matmul(
                        ps[0:B, :],
                        lt,
                        whh_s[:, kb * 2 * G3 : kb * 2 * G3 + G3],
                        start=(kb == 0),
                        stop=(kb == 2 * NS - 1),
                    )
                    pe.matmul(
                        ps[B:P, :],
                        lt,
                        whh_s[:, kb * 2 * G3 + G3 : (kb + 1) * 2 * G3],
                        start=(kb == 0),
                        stop=(kb == 2 * NS - 1),
                        skip_group_check=True,
                    )
                pe.wait_ge(xg_dma[t % XPF], 32 * (t // XPF + 1))
                slot = (t % XPF) * G3
                pe.matmul(
                    ps[:, 0 : 2 * SL],
                    ident_s[:, :],
                    xg_s[:, slot : slot + 2 * SL],
                    start=False,
                    stop=False,
                    skip_group_check=True,
                ).then_inc(xgc_p, 1)
                pe.matmul(
                    ps[0:B, 2 * SL : G3],
                    ones_s[0:1, 0:B],
                    biasn_s[0:1, 0:SL],
                    start=False,
                    stop=False,
                    skip_group_check=True,
                )
                pe.matmul(
                    ps[B:P, 2 * SL : G3],
                    ones_s[0:1, B:P],
                    biasn_s[0:1, SL : 2 * SL],
                    start=False,
                    stop=False,
                    skip_group_check=True,
                ).then_inc(psum_rdy, 1)

                ph1_half(t + LP)

                pe.wait_ge(v2p, t + 1)
                pe.transpose(ps_t[t % 2][:, :], hst_s[:, :], ident_s[:, :]).then_inc(
                    p2v, 1
                )

            # final linear partials over own-direction h
            for d in range(NS):
                pe.wait_ge(rsem[(t_steps - 1) % 2][d], 2 * ((t_steps - 1) // 2 + 1))
            hbo = hb(t_steps)
            for kb in range(2 * NS):
                pe.matmul(
                    ps_lo[:, :],
                    wlin_s[:, kb * SL : (kb + 1) * SL],
                    hbuf[:, hbo + kb * B : hbo + (kb + 1) * B],
                    start=(kb == 0),
                    stop=False,
                )
            pe.matmul(
                ps_lo[:, :],
                blin_s[0:1, :],
                ones_s[0:1, 0:B],
                start=False,
                stop=True,
            ).then_inc(lin_rdy, 1)
            for kb in range(2 * NS):
                mm = pe.matmul(
                    ps_lx[:, :],
                    wlin_s[:, (KT + kb) * SL : (KT + kb + 1) * SL],
                    hbuf[:, hbo + kb * B : hbo + (kb + 1) * B],
                    start=(kb == 0),
                    stop=(kb == 2 * NS - 1),
                )
            mm.then_inc(lin_rdy, 1)

        # ---------------- ACT ----------------------------------------------
        @block.scalar
        def _(a):
            for t in range(t_steps):
                ps = ps_rec[t % 2]
                a.wait_ge(psum_rdy, t + 1)
                a.activation(rz_s[:, 0:SL], ps[:, 0:SL], AFT.Sigmoid).then_inc(
                    a2v_r, 1
                )
                a.activation(
                    rz_s[:, SL : 2 * SL], ps[:, SL : 2 * SL], AFT.Sigmoid
                ).then_inc(a2v_z, 1)
                a.activation(hgn_s[:, :], ps[:, 2 * SL : G3], AFT.Copy).then_inc(
                    a2v_z, 1
                )
                a.wait_ge(v2a_np, t + 1)
                a.activation(n_s[:, :], npre_s[:, :], AFT.Tanh).then_inc(a2v_n, 1)

        # ---------------- DVE ----------------------------------------------
        @block.vector
        def _(v):
            v.memset(hbuf[:, :], 0.0).then_inc(hz_sem, 1)
            v.memset(hst_s[:, :], 0.0).then_inc(hz_sem, 1)
            v.wait_ge(hz_sem, 2)

            def ph1_copy(h):
                if h >= NH:
                    return
                v.wait_ge(p1_rdy, h + 1)
                if h >= 2:
                    v.wait_ge(p1_w, 16 * (h - 1))  # xst slot h-2 written out
                v.tensor_copy(
                    xst[:, (h % 2) * G3 : (h % 2 + 1) * G3], ps_p1[h % 2][:, :]
                ).then_inc(p1_cp, 1)

            for h in range(LP):
                ph1_copy(h)

            for t in range(t_steps):
                ps = ps_rec[t % 2]
                slot = (t % XPF) * G3
                v.wait_ge(a2v_r, t + 1)
                v.wait_ge(a2v_z, 2 * t + 2)
                v.tensor_mul(t1_s[:, :], rz_s[:, 0:SL], hgn_s[:, :]).then_inc(
                    pf_v, 1
                )
                v.wait_ge(pf_v, t + 1)
                v.wait_ge(xg_dma[t % XPF], 32 * (t // XPF + 1))
                v.tensor_add(
                    npre_s[:, :], t1_s[:, :], xg_s[:, slot + 2 * SL : slot + G3]
                ).then_inc(v2a_np, 1)
                v.wait_ge(a2v_n, t + 1)
                if t >= 1:
                    v.wait_ge(v2p, t)
                v.tensor_sub(s1_s[:, :], hst_s[:, :], n_s[:, :]).then_inc(vch, 1)
                v.wait_ge(a2v_z, 2 * t + 1)
                v.wait_ge(vch, 2 * t + 1)
                v.tensor_mul(s2_s[:, :], rz_s[:, SL : 2 * SL], s1_s[:, :]).then_inc(
                    vch, 1
                )
                v.wait_ge(vch, 2 * t + 2)
                v.tensor_add(hst_s[:, :], n_s[:, :], s2_s[:, :]).then_inc(v2p, 1)

                v.wait_ge(p2v, t + 1)
                if t >= 2:
                    for q in range(NQ):
                        v.wait_ge(lsem[q], 16 * (t - 1))  # step t-2 sends done
                v.tensor_copy(
                    tb_s[:, (t % 2) * P : (t % 2) * P + P], ps_t[t % 2][:, :]
                ).then_inc(tdone, 1)

                ph1_copy(t + LP)

            # epilogue: stage cross partial, add received partner partial
            v.wait_ge(lin_rdy, 2)
            v.tensor_copy(linx_s[:, :], ps_lx[:, :]).then_inc(xch_s, 1)
            v.tensor_copy(lino_s[:, :], ps_lo[:, :])
            v.wait_ge(rsem_x, 2)
            v.tensor_add(out_s[:, :], lino_s[:, :], linr_s[:, :]).then_inc(
                fin_sem, 1
            )

        # ---------------- GPSIMD: remote broadcasts -------------------------
        @block.gpsimd
        def _(g):
            g.wait_ge(hz_sem, 1)
            g.remote_sem_update_broadcast(
                remote_sem=bar_sem,
                local_sem=bar_l,
                rdests=[(0, k) for k in range(N)],
            ).then_inc(bar_p, 1)
            g.wait_ge(bar_p, 1)
            g.trigger_dma(count=1)
            g.wait_ge(bar_sem, 16)

            def descgen(s):
                if s >= t_steps:
                    return
                for d in range(NS):
                    rd = [None] * N
                    rd[d] = (0, d)
                    g.remote_dma_broadcast(
                        out_ap=hbuf[:, hb(s + 1) + d * P : hb(s + 1) + (d + 1) * P],
                        in_ap=tb_s[:, (s % 2) * P : (s % 2) * P + P],
                        remote_sem=rsem[s % 2][d],
                        local_sem=lsem[d],
                        rdests=rd,
                        queue_num=d,
                    ).then_inc(prep_q[d], 1)

            for s in range(LEAD):
                descgen(s)
            for t in range(t_steps):
                descgen(t + LEAD)
                g.wait_ge(psum_rdy, t + 1)
                g.wait_ge(tdone, t + 1)
                for q in range(NQ):
                    g.wait_ge(prep_q[q], t + 1)
                    g.trigger_dma(count=1, queue_num=q)

            # epilogue: cross-die partial exchange (relative slot 6)
            rd = [None] * N
            rd[6] = (0, 6)
            g.remote_dma_broadcast(
                out_ap=linr_s[:, :],
                in_ap=linx_s[:, :],
                remote_sem=rsem_x,
                local_sem=lsem_x,
                rdests=rd,
                queue_num=0,
            ).then_inc(prep_x, 1)
            g.wait_ge(xch_s, 1)
            g.wait_ge(prep_x, 1)
            g.trigger_dma(count=1, queue_num=0)

    nc.finalize()
    return nc


# ---- host-side input preparation ---------------------------------------------

BF16_NP = ml_dtypes.bfloat16


def _rows_g(i: int, s: int) -> np.ndarray:
    """Gate rows (r|z|n) of subslice s of in-quad core i, within 3H of one dir."""
    base = np.arange(128 * (2 * i + s), 128 * (2 * i + s) + 128)
    return np.concatenate([base, H + base, 2 * H + base])


def make_core_inputs(r, xT_f, xT_b, Wih_f, Whh_f, bih_f, bhh_f, Wih_b, Whh_b,
                     bih_b, bhh_b, W_lin, b_lin):
    g = r // 4
    i = r % 4
    if g == 0:
        Wih, Whh, bih, bhh, xTs = Wih_f, Whh_f, bih_f, bhh_f, xT_f
    else:
        Wih, Whh, bih, bhh, xTs = Wih_b, Whh_b, bih_b, bhh_b, xT_b
    partner = r + 4 if g == 0 else r - 4

    r0, r1 = _rows_g(i, 0), _rows_g(i, 1)

    def wih_pack():
        o = np.empty((KT, P, 2 * G3), dtype=BF16_NP)
        w0 = np.ascontiguousarray(Wih[r0, :].T)  # [I, 384]
        w1 = np.ascontiguousarray(Wih[r1, :].T)
        for k in range(KT):
            o[k, :, 0:G3] = w0[k * P : (k + 1) * P, :]
            o[k, :, G3 : 2 * G3] = w1[k * P : (k + 1) * P, :]
        return o

    def whh_pack():
        o = np.empty((KT, P, 2 * G3), dtype=BF16_NP)
        w0 = np.ascontiguousarray(Whh[r0, :].T)  # [H, 384]
        w1 = np.ascontiguousarray(Whh[r1, :].T)
        for kb in range(KT):
            d, su = kb // 2, kb % 2
            sj = sigma_in(r, d)
            hsl = slice(128 * (2 * sj + su), 128 * (2 * sj + su) + 128)
            o[kb, :, 0:G3] = w0[hsl, :]
            o[kb, :, G3 : 2 * G3] = w1[hsl, :]
        return o

    def wlin_pack():
        o = np.empty((2 * KT, P, SL), dtype=BF16_NP)
        for sl_i, orow in enumerate((r, partner)):
            wl = np.ascontiguousarray(
                W_lin[orow * SL : (orow + 1) * SL, g * H : (g + 1) * H].T
            )  # [H(own dir), 128]
            for kb in range(KT):
                d, su = kb // 2, kb % 2
                sj = sigma_in(r, d)
                hsl = slice(128 * (2 * sj + su), 128 * (2 * sj + su) + 128)
                o[sl_i * KT + kb] = wl[hsl, :]
        return o

    brz = bih + bhh
    b1 = np.empty((1, 2 * G3), dtype=BF16_NP)
    for c, rows in enumerate((r0, r1)):
        b1[0, c * G3 : c * G3 + 2 * SL] = brz[rows][0 : 2 * SL]
        b1[0, c * G3 + 2 * SL : (c + 1) * G3] = bih[rows][2 * SL : G3]

    bn = np.empty((1, 2 * SL), dtype=BF16_NP)
    bn[0, 0:SL] = bhh[r0][2 * SL : G3]
    bn[0, SL : 2 * SL] = bhh[r1][2 * SL : G3]

    return {
        "xT": xTs,
        "wih": wih_pack(),
        "whh": whh_pack(),
        "wlin": wlin_pack(),
        "bias1": b1,
        "biasn": bn,
        "blin": b_lin[r * SL : (r + 1) * SL].reshape(1, SL).astype(BF16_NP),
        "ident": np.eye(P, dtype=BF16_NP),
        "ones": np.ones((1, P), dtype=BF16_NP),
    }


def make_xT(input_btI: np.ndarray, t_steps: int = T) -> np.ndarray:
    """[B,T,I] -> [NTT, P, KT*P] bf16; tile p partitions = (token 2p | 2p+1)."""
    ntt = t_steps // 2
    xt = np.transpose(input_btI, (1, 0, 2))  # [T, B, I]
    v = xt.reshape(ntt, 2, B, KT, P)
    v = np.transpose(v, (0, 4, 3, 1, 2))  # [tau, i, k, toff, b]
    return np.ascontiguousarray(v.reshape(ntt, P, KT * P)).astype(BF16_NP)


_PROG_CACHE: dict = {}


def get_program(t_steps: int = T):
    if t_steps not in _PROG_CACHE:
        _PROG_CACHE[t_steps] = build_program(t_steps)
    return _PROG_CACHE[t_steps]


def kernel(input, Wih_f, Whh_f, bih_f, bhh_f, Wih_b, Whh_b, bih_b, bhh_b,
           W_lin, b_lin, t_steps: int = T):
    from concourse.bass_utils import run_bass_kernel_spmd

    args = [
        np.asarray(a, dtype=np.float32)
        for a in (Wih_f, Whh_f, bih_f, bhh_f, Wih_b, Whh_b, bih_b, bhh_b,
                  W_lin, b_lin)
    ]
    x = np.asarray(input, dtype=np.float32)[:, :t_steps, :]
    xT_f = make_xT(x, t_steps)
    xT_b = make_xT(x[:, ::-1, :], t_steps)
    nc = get_program(t_steps)
    in_maps = [make_core_inputs(r, xT_f, xT_b, *args) for r in range(N)]
    rr = run_bass_kernel_spmd(nc, in_maps, list(range(N)), **globals().get("RUN_KW", {}))
    res = rr.results
    global LAST_EXEC_NS, LAST_TRACE
    LAST_EXEC_NS = rr.exec_time_ns
    LAST_TRACE = rr.instructions_and_trace
    out = np.concatenate([res[r]["out"].T for r in range(N)], axis=1)
    return np.ascontiguousarray(out).astype(np.float32)
